# revision 1
# baseline (speedup 1.0000x reference)
"""DMoN GCN (3-layer) Trainium2 kernel over 8 NeuronCores — single SPMD launch.

Sharding: core c of 8 handles edges with source in pair-slice s=c//2 (25088
padded nodes, int16-indexable) and target owned by cores of parity g=c%2.
Per layer: PE transform -> pair AllGather of the bf16 node table ->
dma_gather of source rows + PE matmuls against host-shipped weighted one-hot
masks, accumulating exact segment sums in PSUM (duplicate-safe, no scatter)
-> 4-way ReduceScatter(add) -> combine + selu (softmax at the end).
gcn_norm weights (incl. self loops) are baked into the masks on the host.
"""

import numpy as np
import ml_dtypes

BF16 = ml_dtypes.bfloat16

N = 100000
IN_DIM = 256
HID = 128
HID2 = 64
K = 16

NCORE = 8
NOWN = 12544            # 98*128 padded nodes per core
NPAIR = 2 * NOWN        # 25088 source rows per pair  (< 32768 -> int16)
NTGT = 4 * NOWN         # 50176 targets per parity group
NBLK = NTGT // 128      # 392 target blocks
CHUNK = 8192            # slots per dma_gather
SUB = CHUNK // 128

SELU_L = 1.0507009873554805
SELU_A = 1.6732632423543772

_CACHE = {}


def _pad_id(r):
    c = r // 12500
    return c * NOWN + (r - c * 12500)


def _build_plan(edge_index, edge_weight):
    row = np.asarray(edge_index[0], dtype=np.int64)
    col = np.asarray(edge_index[1], dtype=np.int64)
    w = np.asarray(edge_weight, dtype=np.float64)

    deg = np.zeros(N, np.float64)
    np.add.at(deg, col, w)
    deg += 1.0
    dinv = np.where(deg > 0, 1.0 / np.sqrt(deg), 0.0)

    loop = np.arange(N, dtype=np.int64)
    r_all = np.concatenate([row, loop])
    c_all = np.concatenate([col, loop])
    nw = np.concatenate([dinv[row] * w * dinv[col], dinv * dinv])

    rp = _pad_id(r_all)
    cp = _pad_id(c_all)
    src_pair = rp // NPAIR                 # 0..3
    tgt_core = cp // NOWN                  # owner core
    tgt_par = tgt_core % 2                 # parity group
    # local target index within its parity group: rank r = core//2
    tloc = (tgt_core // 2) * NOWN + (cp - tgt_core * NOWN)

    per_core = []
    for c in range(NCORE):
        s, g = c // 2, c % 2
        sel = (src_pair == s) & (tgt_par == g)
        er = (rp[sel] - s * NPAIR).astype(np.int64)
        ec = tloc[sel]
        ew = nw[sel]
        o = np.argsort(ec, kind="stable")
        er, ec, ew = er[o], ec[o], ew[o]
        blk = ec // 128
        cnt = np.bincount(blk, minlength=NBLK)
        pcnt = np.maximum(((cnt + 127) // 128) * 128, 128)
        per_core.append((er, ec, ew, blk, cnt, pcnt))

    # shared program structure: per-block sub-chunk counts = max across cores
    nsub_blk = np.stack([p[5] // 128 for p in per_core]).max(axis=0)
    tot_sub = int(nsub_blk.sum())
    nchunk = (tot_sub + SUB - 1) // SUB
    pad_sub = nchunk * SUB - tot_sub
    nsub_blk = nsub_blk.copy()
    nsub_blk[-1] += pad_sub
    tot_sub = nchunk * SUB
    sub_off = np.concatenate([[0], np.cumsum(nsub_blk)])[:-1]
    blk_of_sub = np.repeat(np.arange(NBLK), nsub_blk)
    start_of_sub = np.zeros(tot_sub, bool)
    start_of_sub[sub_off] = True

    idx_all, msk_all = [], []
    ii = np.arange(CHUNK)
    for c in range(NCORE):
        er, ec, ew, blk, cnt, _ = per_core[c]
        nslot = tot_sub * 128
        sidx = np.zeros(nslot, np.int16)
        stgt = np.zeros(nslot, np.int64)
        sw = np.zeros(nslot)
        cnt_off = np.concatenate([[0], np.cumsum(cnt)])[:-1]
        pos = sub_off[blk] * 128 + (np.arange(len(ec)) - cnt_off[blk])
        sidx[pos] = er.astype(np.int16)
        stgt[pos] = ec - blk * 128
        sw[pos] = ew
        idx_w = np.zeros((nchunk, 128, CHUNK // 16), np.int16)
        for ch in range(nchunk):
            seg = sidx[ch * CHUNK:(ch + 1) * CHUNK]
            t16 = np.zeros((16, CHUNK // 16), np.int16)
            t16[ii % 16, ii // 16] = seg
            idx_w[ch] = np.tile(t16, (8, 1))
        masks = np.zeros((tot_sub, 128, 128), np.float32)
        sl = np.arange(nslot)
        masks[sl // 128, sl % 128, stgt] = sw
        idx_all.append(idx_w)
        msk_all.append(masks.astype(BF16))

    return dict(nchunk=nchunk, blk_of_sub=blk_of_sub,
                start_of_sub=start_of_sub, idx=idx_all, msk=msk_all)


def _build_program(nchunk, blk_of_sub, start_of_sub):
    import concourse.bacc as bacc
    import concourse.mybir as mybir
    from concourse import tile

    nc = bacc.Bacc("TRN2", target_bir_lowering=False, debug=False,
                   num_devices=NCORE)
    f32, bf16, i16 = mybir.dt.float32, mybir.dt.bfloat16, mybir.dt.int16
    AL = mybir.AluOpType
    TOT_SUB = nchunk * SUB
    FD = [HID, HID2, K]
    NT = NOWN // 128

    t_x = nc.dram_tensor("x", [NOWN, IN_DIM], f32, kind="ExternalInput")
    t_idx1 = nc.dram_tensor("idx", [nchunk, 128, CHUNK // 16], i16,
                            kind="ExternalInput")
    t_msk1 = nc.dram_tensor("msk", [TOT_SUB, 128, 128], bf16,
                            kind="ExternalInput")
    t_idx = [t_idx1] * 3
    t_msk = [t_msk1] * 3
    t_W0 = nc.dram_tensor("W0p", [2, 128, HID], f32, kind="ExternalInput")
    t_P0 = nc.dram_tensor("P0p", [2, 128, HID], f32, kind="ExternalInput")
    t_W1 = nc.dram_tensor("W1p", [1, 128, HID2], f32, kind="ExternalInput")
    t_P1 = nc.dram_tensor("P1p", [1, 128, HID2], f32, kind="ExternalInput")
    t_W2 = nc.dram_tensor("W2p", [1, 128, K], f32, kind="ExternalInput")
    t_b0 = nc.dram_tensor("b0r", [128, HID], f32, kind="ExternalInput")
    t_b1 = nc.dram_tensor("b1r", [128, HID2], f32, kind="ExternalInput")
    t_b2 = nc.dram_tensor("b2r", [128, K], f32, kind="ExternalInput")
    t_id = nc.dram_tensor("ident", [128, 128], f32, kind="ExternalInput")
    t_out = nc.dram_tensor("out", [NOWN, K], f32, kind="ExternalOutput")

    with tile.TileContext(nc) as tc:
        with (
            tc.tile_pool(name="dram", bufs=1, space="DRAM") as dpool,
            tc.tile_pool(name="wts", bufs=1) as wpool,
            tc.tile_pool(name="work", bufs=4) as pool,
            tc.tile_pool(name="ps", bufs=2, space="PSUM") as ppool,
            tc.tile_pool(name="acc", bufs=2, space="PSUM") as apool,
        ):
            d_tab_own = [dpool.tile([NOWN, 128], bf16, tag=f"tabown{l}", name=f"tabown{l}")
                         for l in range(3)]
            d_tab_pair = [dpool.tile([NPAIR, 128], bf16, tag=f"tabpair{l}", name=f"tabpair{l}")
                          for l in range(3)]
            d_sk = [dpool.tile([NOWN, FD[l]], f32, tag=f"sk{l}", name=f"sk{l}")
                    for l in range(2)]
            d_part = [dpool.tile([NTGT, FD[l]], f32, tag=f"part{l}", name=f"part{l}")
                      for l in range(3)]
            d_rs = [dpool.tile([NOWN, FD[l]], f32, tag=f"rs{l}", name=f"rs{l}")
                    for l in range(3)]
            d_h = [dpool.tile([NOWN, FD[l]], f32, tag=f"h{l}", name=f"h{l}")
                   for l in range(2)]

            idt = wpool.tile([128, 128], f32)
            nc.sync.dma_start(idt[:], t_id[:])
            def wload(tname, src_t, n, fdim):
                ts = []
                for j in range(n):
                    wt = wpool.tile([128, fdim], f32, name=f"{tname}_{j}")
                    nc.sync.dma_start(wt[:], src_t[j])
                    ts.append(wt)
                return ts
            wW0 = wload("wW0", t_W0, 2, HID)
            wP0 = wload("wP0", t_P0, 2, HID)
            wW1 = wload("wW1", t_W1, 1, HID2)
            wP1 = wload("wP1", t_P1, 1, HID2)
            wW2 = wload("wW2", t_W2, 1, K)
            wb0 = wpool.tile([128, HID], f32)
            nc.sync.dma_start(wb0[:], t_b0[:])
            wb1 = wpool.tile([128, HID2], f32)
            nc.sync.dma_start(wb1[:], t_b1[:])
            wb2 = wpool.tile([128, K], f32)
            nc.sync.dma_start(wb2[:], t_b2[:])

            def transform(l, src_dram, fin, fout, Wt, Pt, skt, bias):
                ncin = (fin + 127) // 128
                for t in range(NT):
                    xin = pool.tile([128, ncin * 128], f32, tag="xin")
                    if fin % 128:
                        nc.gpsimd.memset(xin[:], 0.0)
                    nc.sync.dma_start(xin[:, :fin],
                                      src_dram[t * 128:(t + 1) * 128, :])
                    xT = pool.tile([128, ncin * 128], f32, tag="xT")
                    for j in range(ncin):
                        pt = ppool.tile([128, 128], f32, tag="ptr")
                        nc.tensor.transpose(
                            pt[:], xin[:, j * 128:(j + 1) * 128], idt[:])
                        nc.vector.tensor_copy(
                            xT[:, j * 128:(j + 1) * 128], pt[:])
                    pm = ppool.tile([128, fout], f32, tag="pmm")
                    for j in range(ncin):
                        nc.tensor.matmul(pm[:], xT[:, j * 128:(j + 1) * 128],
                                         Wt[j][:], start=(j == 0),
                                         stop=(j == ncin - 1))
                    tb = pool.tile([128, 128], bf16, tag="tabtile")
                    if fout < 128:
                        nc.gpsimd.memset(tb[:], 0.0)
                    nc.vector.tensor_copy(tb[:, :fout], pm[:])
                    nc.sync.dma_start(
                        d_tab_own[l][t * 128:(t + 1) * 128, :], tb[:])
                    if Pt is not None:
                        ps = ppool.tile([128, fout], f32, tag="psk")
                        for j in range(ncin):
                            nc.tensor.matmul(ps[:],
                                             xT[:, j * 128:(j + 1) * 128],
                                             Pt[j][:], start=(j == 0),
                                             stop=(j == ncin - 1))
                        sk = pool.tile([128, fout], f32, tag="sktile")
                        nc.vector.tensor_tensor(sk[:], ps[:], bias[:, :fout],
                                                AL.add)
                        nc.sync.dma_start(
                            skt[t * 128:(t + 1) * 128, :], sk[:])

            def aggregate(l, fout):
                acc = None
                sub = 0
                for ch in range(nchunk):
                    it = pool.tile([128, CHUNK // 16], i16, tag="idxt")
                    nc.sync.dma_start(it[:], t_idx[l][ch])
                    g = pool.tile([128, SUB, 128], bf16, tag="gath")
                    nc.gpsimd.dma_gather(g[:], d_tab_pair[l][:], it[:],
                                         CHUNK, CHUNK, 128,
                                         single_packet=False)
                    mk = pool.tile([128, SUB, 128], bf16, tag="maskt")
                    nc.sync.dma_start(
                        mk[:],
                        t_msk[l][ch * SUB:(ch + 1) * SUB].rearrange(
                            "s p t -> p s t"))
                    for j in range(SUB):
                        st = bool(start_of_sub[sub])
                        if st:
                            acc = apool.tile([128, fout], f32, tag="accps",
                                             name=f"acc{l}_{sub}")
                        last = (sub == TOT_SUB - 1) or bool(
                            start_of_sub[sub + 1])
                        nc.tensor.matmul(acc[:], mk[:, j, :],
                                         g[:, j, :fout], start=st, stop=last)
                        if last:
                            blk = int(blk_of_sub[sub])
                            ev = pool.tile([128, fout], f32, tag="ev")
                            nc.vector.tensor_copy(ev[:], acc[:])
                            nc.sync.dma_start(
                                d_part[l][blk * 128:(blk + 1) * 128, :],
                                ev[:])
                        sub += 1

            def combine(l, fout, skt, h_out):
                for t in range(NT):
                    zz = pool.tile([128, fout], f32, tag="z")
                    nc.sync.dma_start(zz[:],
                                      d_rs[l][t * 128:(t + 1) * 128, :])
                    if l < 2:
                        sk = pool.tile([128, fout], f32, tag="skld")
                        nc.sync.dma_start(sk[:],
                                          skt[t * 128:(t + 1) * 128, :])
                        nc.vector.tensor_tensor(zz[:], zz[:], sk[:], AL.add)
                    else:
                        nc.vector.tensor_tensor(zz[:], zz[:], wb2[:, :fout],
                                                AL.add)
                    mn = pool.tile([128, fout], f32, tag="smn")
                    nc.vector.tensor_scalar_min(mn[:], zz[:], 0.0)
                    ex = pool.tile([128, fout], f32, tag="sex")
                    nc.scalar.activation(ex[:], mn[:],
                                         mybir.ActivationFunctionType.Exp)
                    nc.vector.tensor_scalar(ex[:], ex[:], SELU_L * SELU_A,
                                            -SELU_L * SELU_A, AL.mult,
                                            AL.add)
                    nc.vector.tensor_scalar_max(zz[:], zz[:], 0.0)
                    nc.vector.tensor_scalar(zz[:], zz[:], SELU_L, None,
                                            AL.mult)
                    nc.vector.tensor_tensor(zz[:], zz[:], ex[:], AL.add)
                    if l < 2:
                        nc.sync.dma_start(
                            h_out[t * 128:(t + 1) * 128, :], zz[:])
                    else:
                        mx = pool.tile([128, 1], f32, tag="smx")
                        nc.vector.tensor_reduce(mx[:], zz[:], mybir.AxisListType.X, AL.max)
                        nc.vector.tensor_scalar(zz[:], zz[:], mx[:], None,
                                                AL.subtract)
                        nc.scalar.activation(
                            zz[:], zz[:], mybir.ActivationFunctionType.Exp)
                        sm = pool.tile([128, 1], f32, tag="ssm")
                        nc.vector.tensor_reduce(sm[:], zz[:], mybir.AxisListType.X, AL.add)
                        rc = pool.tile([128, 1], f32, tag="src")
                        nc.vector.reciprocal(rc[:], sm[:])
                        nc.vector.tensor_scalar(zz[:], zz[:], rc[:], None,
                                                AL.mult)
                        nc.sync.dma_start(
                            t_out[t * 128:(t + 1) * 128, :], zz[:])

            srcs = [t_x, d_h[0], d_h[1]]
            fins = [IN_DIM, HID, HID2]
            Ws = [wW0, wW1, wW2]
            Ps = [wP0, wP1, None]
            sks = [d_sk[0], d_sk[1], None]
            bs = [wb0, wb1, wb2]
            for l in range(3):
                transform(l, srcs[l], fins[l], FD[l], Ws[l], Ps[l],
                          sks[l], bs[l])
                nc.gpsimd.collective_compute(
                    "AllGather", mybir.AluOpType.bypass,
                    replica_groups=[[0, 1], [2, 3], [4, 5], [6, 7]],
                    ins=[d_tab_own[l][:].opt()],
                    outs=[d_tab_pair[l][:].opt()])
                aggregate(l, FD[l])
                nc.gpsimd.collective_compute(
                    "ReduceScatter", mybir.AluOpType.add,
                    replica_groups=[[0, 2, 4, 6], [1, 3, 5, 7]],
                    ins=[d_part[l][:].opt()], outs=[d_rs[l][:].opt()])
                combine(l, FD[l], sks[l], d_h[l] if l < 2 else None)
    nc.compile()
    return nc


def _get_compiled(inputs):
    k = "prog"
    if k not in _CACHE:
        plan = _build_plan(inputs["edge_index"], inputs["edge_weight"])
        nc = _build_program(plan["nchunk"], plan["blk_of_sub"],
                            plan["start_of_sub"])
        _CACHE[k] = (plan, nc)
    return _CACHE[k]


def kernel(_trace=False, **inputs):
    from concourse.bass_utils import run_bass_kernel_spmd

    plan, nc = _get_compiled(inputs)

    x = np.asarray(inputs["x"], np.float32)
    xpad = np.zeros((NCORE, NOWN, IN_DIM), np.float32)
    for c in range(NCORE):
        xpad[c, :12500] = x[c * 12500:(c + 1) * 12500]

    def wchunks(W, n):
        out = np.zeros((n, 128, W.shape[1]), np.float32)
        for j in range(n):
            out[j, :min(128, W.shape[0] - j * 128)] = \
                W[j * 128:(j + 1) * 128]
        return out

    W0 = np.asarray(inputs["W0"], np.float32)
    P0 = np.asarray(inputs["P0w"], np.float32)
    W1p = wchunks(np.asarray(inputs["W1"], np.float32), 1)
    P1p = wchunks(np.asarray(inputs["P1w"], np.float32), 1)
    W2p = wchunks(np.asarray(inputs["W2"], np.float32), 1)
    b0r = np.tile((np.asarray(inputs["b0"]) + np.asarray(inputs["P0b"]))
                  .astype(np.float32), (128, 1))
    b1r = np.tile((np.asarray(inputs["b1"]) + np.asarray(inputs["P1b"]))
                  .astype(np.float32), (128, 1))
    b2r = np.tile(np.asarray(inputs["b2"]).astype(np.float32), (128, 1))

    in_maps = []
    for c in range(NCORE):
        in_maps.append({
            "x": xpad[c],
            "idx": plan["idx"][c], "msk": plan["msk"][c],
            "W0p": wchunks(W0, 2), "P0p": wchunks(P0, 2),
            "W1p": W1p, "P1p": P1p, "W2p": W2p,
            "b0r": b0r, "b1r": b1r, "b2r": b2r,
            "ident": np.eye(128, dtype=np.float32),
        })
    res = run_bass_kernel_spmd(nc, in_maps, core_ids=list(range(NCORE)),
                               trace=_trace)
    if _trace:
        kernel.last_exec_ns = res.exec_time_ns
    out = np.zeros((N, K), np.float32)
    for c in range(NCORE):
        out[c * 12500:(c + 1) * 12500] = res.results[c]["out"][:12500]
    return out



# revision 6
# speedup vs baseline: 1.1498x; 1.1498x over previous
"""DMoN GCN (3-layer) Trainium2 kernel over 8 NeuronCores — single SPMD launch.

v2 design: core c owns target nodes [c*12544, (c+1)*12544) (98 blocks of
128).  Per layer: PE transform of own nodes -> segmented 8-way AllGather of
the bf16 node table -> aggregation by block-group x source-pair:
dma_gather of source rows (int16 indices local to a 25088-row pair slice),
DVE generates weighted one-hot masks on-chip from resident metadata
(iota==target fused with x weight), PE accumulates exact block segment sums
in PSUM, DVE folds them into an SBUF accumulator.  Per-block combine
(+skip+bias, selu) feeds the next layer's transform directly (interleaved
with aggregation so the next AllGather overlaps gather emission).  Final
layer: transpose + softmax.  gcn_norm weights (incl. self loops) are
computed on the host and shipped as per-slot metadata.
"""

import numpy as np
import ml_dtypes

BF16 = ml_dtypes.bfloat16

N = 100000
IN_DIM = 256
HID = 128
HID2 = 64
K = 16

NCORE = 8
NOWN = 12544             # 98*128 padded nodes per core
NPAIR = 2 * NOWN         # 25088 rows per pair slice (< 32768 -> int16)
NALL = 8 * NOWN
NBLK = 98                # target blocks per core
NPH = 4                  # source pair phases
BG = 7                   # blocks per gather group
NGRP = NBLK // BG        # 14
SEG = [0, NBLK]   # AllGather segments (block ranges); BIR requires
                  # contiguous collective outputs -> single segment

FOUT = [HID, HID2, K]
FIN = [IN_DIM, HID, HID2]

SELU_L = 1.0507009873554805
SELU_A = 1.6732632423543772

GATHER_SINGLE_PACKET = False
NUM_SWDGE_QUEUES = 4

_CACHE = {}


def _pad_id(r):
    c = r // 12500
    return c * NOWN + (r - c * 12500)


def _build_plan(edge_index, edge_weight):
    row = np.asarray(edge_index[0], dtype=np.int64)
    col = np.asarray(edge_index[1], dtype=np.int64)
    w = np.asarray(edge_weight, dtype=np.float64)

    deg = np.zeros(N, np.float64)
    np.add.at(deg, col, w)
    deg += 1.0
    dinv = 1.0 / np.sqrt(deg)

    loop = np.arange(N, dtype=np.int64)
    r_all = np.concatenate([row, loop])
    c_all = np.concatenate([col, loop])
    nw = np.concatenate([dinv[row] * w * dinv[col], dinv * dinv])

    rp = _pad_id(r_all)
    cp = _pad_id(c_all)
    tcore = cp // NOWN

    per_core = []
    cnts = np.zeros((NCORE, NBLK, NPH), np.int64)
    for c in range(NCORE):
        sel = tcore == c
        tp = cp[sel] - c * NOWN
        blk = tp // 128
        toff = tp % 128
        pair = rp[sel] // NPAIR
        sloc = rp[sel] - pair * NPAIR
        ew = nw[sel]
        per_core.append((blk, toff, pair, sloc, ew))
        np.add.at(cnts, (c, blk, pair), 1)

    nsub = np.maximum(
        (cnts.max(axis=0) + 127) // 128, 1).astype(np.int64)  # [NBLK, NPH]

    # global sub order: group-major, pair, block, sub-within-run
    sub_blocks, sub_start, sub_stop = [], [], []
    chunk_list = []                      # (pair, sub_off, nsubs) per gather
    run_sub_off = np.zeros((NBLK, NPH), np.int64)
    S = 0
    for g in range(NGRP):
        bs = range(g * BG, min((g + 1) * BG, NBLK))
        for p in range(NPH):
            ch_off = S
            for b in bs:
                run_sub_off[b, p] = S
                k = int(nsub[b, p])
                for i in range(k):
                    sub_blocks.append(b)
                    sub_start.append(i == 0)
                    sub_stop.append(i == k - 1)
                    S += 1
            chunk_list.append((p, ch_off, S - ch_off))
    TOT = S

    NRUN = NBLK * NPH
    run_id = np.arange(NRUN).reshape(NBLK, NPH)
    idx_all, meta_all = [], []
    for c in range(NCORE):
        blk, toff, pair, sloc, ew = per_core[c]
        rid = run_id[blk, pair]
        order = np.argsort(rid, kind="stable")
        rid_s = rid[order]
        rc_cnt = np.bincount(rid_s, minlength=NRUN)
        first = np.concatenate([[0], np.cumsum(rc_cnt)])[:-1]
        within = np.arange(len(rid_s)) - first[rid_s]
        pos = run_sub_off.reshape(-1)[rid_s] * 128 + within
        assert (within < nsub.reshape(-1)[rid_s] * 128).all()

        sidx = np.zeros(TOT * 128, np.int16)
        stof = np.zeros(TOT * 128, np.float32)
        sw = np.zeros(TOT * 128, np.float32)
        sidx[pos] = sloc[order].astype(np.int16)
        stof[pos] = toff[order]
        sw[pos] = ew[order]

        # idx wrapped in 16 partitions, replicated x8 -> [128, TOT*8]
        ii = np.arange(TOT * 128)
        t16 = np.zeros((16, TOT * 8), np.int16)
        t16[ii % 16, ii // 16] = sidx
        idx_all.append(np.tile(t16, (8, 1)))

        meta = np.zeros((128, TOT, 2), np.float32)
        sl = np.arange(TOT * 128)
        meta[sl % 128, sl // 128, 0] = stof
        meta[sl % 128, sl // 128, 1] = sw
        meta_all.append(meta.reshape(128, TOT * 2))

    return dict(TOT=TOT, chunk_list=chunk_list,
                sub_blocks=np.array(sub_blocks),
                sub_start=np.array(sub_start), sub_stop=np.array(sub_stop),
                nsub=nsub, idx=idx_all, meta=meta_all)


def _build_program(plan):
    import concourse.bacc as bacc
    import concourse.mybir as mybir
    from concourse import tile

    nc = bacc.Bacc("TRN2", target_bir_lowering=False, debug=False,
                   num_devices=NCORE, num_swdge_queues=NUM_SWDGE_QUEUES)
    f32, bf16, i16 = mybir.dt.float32, mybir.dt.bfloat16, mybir.dt.int16
    AL = mybir.AluOpType
    ACT_EXP = mybir.ActivationFunctionType.Exp
    AXX = mybir.AxisListType.X

    TOT = plan["TOT"]
    chunk_list = plan["chunk_list"]
    sub_blocks = plan["sub_blocks"]
    sub_start = plan["sub_start"]
    sub_stop = plan["sub_stop"]

    t_x = nc.dram_tensor("x", [NOWN, IN_DIM], f32, kind="ExternalInput")
    t_idx = nc.dram_tensor("idxs", [128, TOT * 8], i16, kind="ExternalInput")
    t_meta = nc.dram_tensor("meta", [128, TOT * 2], f32,
                            kind="ExternalInput")
    t_W0 = nc.dram_tensor("W0p", [2, 128, HID], f32, kind="ExternalInput")
    t_P0 = nc.dram_tensor("P0p", [2, 128, HID], f32, kind="ExternalInput")
    t_W1 = nc.dram_tensor("W1", [HID, HID2], f32, kind="ExternalInput")
    t_P1 = nc.dram_tensor("P1", [HID, HID2], f32, kind="ExternalInput")
    t_W2 = nc.dram_tensor("W2", [HID2, K], f32, kind="ExternalInput")
    t_bc0 = nc.dram_tensor("bc0", [HID, 1], f32, kind="ExternalInput")
    t_bc1 = nc.dram_tensor("bc1", [HID2, 1], f32, kind="ExternalInput")
    t_bc2 = nc.dram_tensor("bc2", [K, 1], f32, kind="ExternalInput")
    t_id = nc.dram_tensor("ident", [128, 128], f32, kind="ExternalInput")
    t_iota = nc.dram_tensor("iota", [128, 128], f32, kind="ExternalInput")
    t_out = nc.dram_tensor("out", [NOWN, K], f32, kind="ExternalOutput")

    with tile.TileContext(nc) as tc:
        with (
            tc.tile_pool(name="dram", bufs=1, space="DRAM") as dpool,
            tc.tile_pool(name="wts", bufs=1) as wpool,
            tc.tile_pool(name="work", bufs=3) as pool,
            tc.tile_pool(name="msk", bufs=6) as mpool,
            tc.tile_pool(name="ps", bufs=2, space="PSUM") as ppool,
            tc.tile_pool(name="acc", bufs=2, space="PSUM") as apool,
        ):
            d_tab = [dpool.tile([NOWN, 128], bf16, tag=f"tab{l}",
                                name=f"tab{l}") for l in range(3)]
            d_all = [dpool.tile([NALL, 128], bf16, tag=f"all{l}",
                                name=f"all{l}") for l in range(3)]
            d_skT = [dpool.tile([FOUT[l], NOWN], f32, tag=f"skT{l}",
                                name=f"skT{l}") for l in range(2)]

            idt = wpool.tile([128, 128], f32)
            nc.sync.dma_start(idt[:], t_id[:])
            iot = wpool.tile([128, 128], f32)
            nc.sync.dma_start(iot[:], t_iota[:])
            meta_sb = wpool.tile([128, TOT * 2], f32)
            nc.sync.dma_start(meta_sb[:], t_meta[:])
            sb_acc = wpool.tile([128, NOWN], f32)

            wW0, wP0 = [], []
            for j in range(2):
                wt = wpool.tile([128, HID], f32, name=f"w0_{j}")
                nc.sync.dma_start(wt[:], t_W0[j])
                wW0.append(wt)
                pt_ = wpool.tile([128, HID], f32, name=f"p0_{j}")
                nc.sync.dma_start(pt_[:], t_P0[j])
                wP0.append(pt_)
            wW1 = wpool.tile([HID, HID2], f32)
            nc.sync.dma_start(wW1[:], t_W1[:])
            wP1 = wpool.tile([HID, HID2], f32)
            nc.sync.dma_start(wP1[:], t_P1[:])
            wW2 = wpool.tile([HID2, K], f32)
            nc.sync.dma_start(wW2[:], t_W2[:])
            wb0 = wpool.tile([HID, 1], f32)
            nc.sync.dma_start(wb0[:], t_bc0[:])
            wb1 = wpool.tile([HID2, 1], f32)
            nc.sync.dma_start(wb1[:], t_bc1[:])
            wb2 = wpool.tile([K, 1], f32)
            nc.sync.dma_start(wb2[:], t_bc2[:])

            Ws = [wW0, [wW1], [wW2]]
            Ps = [wP0, [wP1], None]
            Bs = [wb0, wb1, wb2]

            def fire_ag(l, s):
                rs, re = SEG[s] * 128, SEG[s + 1] * 128
                outs = d_all[l][:].rearrange("(c r) f -> c r f",
                                             c=NCORE)[:, rs:re, :]
                nc.gpsimd.collective_compute(
                    "AllGather", AL.bypass,
                    replica_groups=[[0, 1, 2, 3, 4, 5, 6, 7]],
                    ins=[d_tab[l][rs:re, :].opt()], outs=[outs.opt()])

            def transform_tile(l, t, xTj):
                fout = FOUT[l]
                pm = ppool.tile([128, fout], f32, tag="pmm")
                nj = len(xTj)
                for j, xj in enumerate(xTj):
                    nc.tensor.matmul(pm[:], xj, Ws[l][j][:],
                                     start=(j == 0), stop=(j == nj - 1))
                tb = pool.tile([128, 128], bf16, tag="tab")
                if fout < 128:
                    nc.vector.memset(tb[:], 0.0)
                nc.vector.tensor_copy(tb[:, :fout], pm[:])
                nc.sync.dma_start(d_tab[l][t * 128:(t + 1) * 128, :], tb[:])
                if l < 2:
                    ps = ppool.tile([fout, 128], f32, tag="psk")
                    for j, xj in enumerate(xTj):
                        nc.tensor.matmul(ps[:], Ps[l][j][:], xj,
                                         start=(j == 0), stop=(j == nj - 1))
                    sk = pool.tile([fout, 128], f32, tag="skw")
                    nc.vector.tensor_scalar(sk[:], ps[:], Bs[l][:], None,
                                            AL.add)
                    nc.sync.dma_start(d_skT[l][:, t * 128:(t + 1) * 128],
                                      sk[:])

            def combine_block(l, b):
                fout = FOUT[l]
                sl = sb_acc[:fout, b * 128:(b + 1) * 128]
                zz = pool.tile([fout, 128], f32, tag=f"zz{l}")
                if l < 2:
                    skt = pool.tile([fout, 128], f32, tag="skld")
                    nc.sync.dma_start(skt[:],
                                      d_skT[l][:, b * 128:(b + 1) * 128])
                    nc.vector.tensor_tensor(zz[:], sl, skt[:], AL.add)
                else:
                    nc.vector.tensor_scalar(zz[:], sl, Bs[2][:], None,
                                            AL.add)
                ex = pool.tile([fout, 128], f32, tag=f"ex{l}")
                nc.vector.tensor_scalar_min(ex[:], zz[:], 0.0)
                e2 = pool.tile([fout, 128], f32, tag=f"e2{l}")
                nc.scalar.activation(e2[:], ex[:], ACT_EXP)
                nc.vector.tensor_scalar(e2[:], e2[:], SELU_L * SELU_A,
                                        -SELU_L * SELU_A, AL.mult, AL.add)
                nc.vector.tensor_scalar(zz[:], zz[:], 0.0, SELU_L,
                                        AL.max, AL.mult)
                nc.vector.tensor_tensor(zz[:], zz[:], e2[:], AL.add)
                if l < 2:
                    transform_tile(l + 1, b, [zz[:]])
                else:
                    pt2 = ppool.tile([128, K], f32, tag="psk")
                    nc.tensor.matmul(pt2[:], zz[:], idt[:K, :K],
                                     start=True, stop=True)
                    sm = pool.tile([128, K], f32, tag="sms")
                    mx = pool.tile([128, 1], f32, tag="mx")
                    nc.vector.tensor_reduce(mx[:], pt2[:], AXX, AL.max)
                    nc.vector.tensor_scalar(sm[:], pt2[:], mx[:], None,
                                            AL.subtract)
                    nc.scalar.activation(sm[:], sm[:], ACT_EXP)
                    s2 = pool.tile([128, 1], f32, tag="s2")
                    nc.vector.tensor_reduce(s2[:], sm[:], AXX, AL.add)
                    rc = pool.tile([128, 1], f32, tag="rc")
                    nc.vector.reciprocal(rc[:], s2[:])
                    nc.vector.tensor_scalar(sm[:], sm[:], rc[:], None,
                                            AL.mult)
                    nc.sync.dma_start(t_out[b * 128:(b + 1) * 128, :], sm[:])

            # ---- layer 0 transform (standalone) + segmented AG(0) ----
            seg_after_tile = {SEG[s + 1] - 1: s for s in range(len(SEG) - 1)}
            for t in range(NBLK):
                xin = pool.tile([128, IN_DIM], f32, tag="xin")
                nc.sync.dma_start(xin[:], t_x[t * 128:(t + 1) * 128, :])
                xT = pool.tile([128, IN_DIM], f32, tag="xT")
                for j in range(2):
                    ptr = ppool.tile([128, 128], f32, tag="ptr")
                    nc.tensor.transpose(ptr[:],
                                        xin[:, j * 128:(j + 1) * 128],
                                        idt[:])
                    nc.vector.tensor_copy(xT[:, j * 128:(j + 1) * 128],
                                          ptr[:])
                transform_tile(0, t, [xT[:, 0:128], xT[:, 128:256]])
                if t in seg_after_tile:
                    fire_ag(0, seg_after_tile[t])

            seg_after_group = {}
            for s in range(len(SEG) - 1):
                g = (SEG[s + 1] + BG - 1) // BG - 1
                seg_after_group.setdefault(g, []).append(s)

            # ---- layers ----
            for l in range(3):
                fout = FOUT[l]
                acc = None
                for g in range(NGRP):
                    for p in range(NPH):
                        pair, ch_off, kk = chunk_list[g * NPH + p]
                        it = pool.tile([128, kk * 8], i16, tag="idxt")
                        nc.sync.dma_start(
                            it[:], t_idx[:, ch_off * 8:(ch_off + kk) * 8])
                        gth = pool.tile([128, kk, 128], bf16, tag="gath")
                        qn = (g * NPH + p) % 4
                        nc.gpsimd.dma_gather(
                            gth[:], d_all[l][pair * NPAIR:(pair + 1) * NPAIR,
                                             :],
                            it[:], kk * 128, kk * 128, 128,
                            single_packet=GATHER_SINGLE_PACKET,
                            queue_num=qn)
                        for j in range(kk):
                            sub = ch_off + j
                            if sub_start[sub]:
                                acc = apool.tile([fout, 128], f32,
                                                 tag="accp",
                                                 name=f"acc{l}_{sub}")
                            mk = mpool.tile([128, 128], bf16, tag="mask")
                            nc.vector.tensor_scalar(
                                mk[:], iot[:],
                                meta_sb[:, 2 * sub:2 * sub + 1],
                                meta_sb[:, 2 * sub + 1:2 * sub + 2],
                                AL.is_equal, AL.mult)
                            nc.tensor.matmul(acc[:], gth[:, j, :fout],
                                             mk[:],
                                             start=bool(sub_start[sub]),
                                             stop=bool(sub_stop[sub]))
                            if sub_stop[sub]:
                                b = int(sub_blocks[sub])
                                dst = sb_acc[:fout,
                                             b * 128:(b + 1) * 128]
                                if p == 0:
                                    nc.vector.tensor_copy(dst, acc[:])
                                else:
                                    nc.vector.tensor_tensor(dst, dst,
                                                            acc[:], AL.add)
                    for b in range(g * BG, min((g + 1) * BG, NBLK)):
                        combine_block(l, b)
                    if l < 2 and g in seg_after_group:
                        for s in seg_after_group[g]:
                            fire_ag(l + 1, s)
    nc.compile()
    return nc


def _get_compiled(inputs):
    k = "prog"
    if k not in _CACHE:
        plan = _build_plan(inputs["edge_index"], inputs["edge_weight"])
        nc = _build_program(plan)
        _CACHE[k] = (plan, nc)
    return _CACHE[k]


def kernel(_trace=False, **inputs):
    from concourse.bass_utils import run_bass_kernel_spmd

    plan, nc = _get_compiled(inputs)

    x = np.asarray(inputs["x"], np.float32)
    xpad = np.zeros((NCORE, NOWN, IN_DIM), np.float32)
    for c in range(NCORE):
        xpad[c, :12500] = x[c * 12500:(c + 1) * 12500]

    def wchunks(W, n):
        out = np.zeros((n, 128, W.shape[1]), np.float32)
        for j in range(n):
            out[j, :min(128, W.shape[0] - j * 128)] = \
                W[j * 128:(j + 1) * 128]
        return out

    W0p = wchunks(np.asarray(inputs["W0"], np.float32), 2)
    P0p = wchunks(np.asarray(inputs["P0w"], np.float32), 2)
    W1 = np.asarray(inputs["W1"], np.float32)
    P1 = np.asarray(inputs["P1w"], np.float32)
    W2 = np.asarray(inputs["W2"], np.float32)
    bc0 = (np.asarray(inputs["b0"]) + np.asarray(inputs["P0b"])) \
        .astype(np.float32).reshape(HID, 1)
    bc1 = (np.asarray(inputs["b1"]) + np.asarray(inputs["P1b"])) \
        .astype(np.float32).reshape(HID2, 1)
    bc2 = np.asarray(inputs["b2"]).astype(np.float32).reshape(K, 1)
    ident = np.eye(128, dtype=np.float32)
    iota = np.tile(np.arange(128, dtype=np.float32), (128, 1))

    in_maps = []
    for c in range(NCORE):
        in_maps.append({
            "x": xpad[c],
            "idxs": plan["idx"][c], "meta": plan["meta"][c],
            "W0p": W0p, "P0p": P0p, "W1": W1, "P1": P1, "W2": W2,
            "bc0": bc0, "bc1": bc1, "bc2": bc2,
            "ident": ident, "iota": iota,
        })
    res = run_bass_kernel_spmd(nc, in_maps, core_ids=list(range(NCORE)),
                               trace=_trace)
    if _trace:
        kernel.last_exec_ns = res.exec_time_ns
        kernel.last_profile_json = res.profile_json
    out = np.zeros((N, K), np.float32)
    for c in range(NCORE):
        out[c * 12500:(c + 1) * 12500] = res.results[c]["out"][:12500]
    return out


# revision 7
# speedup vs baseline: 1.7858x; 1.5532x over previous
"""DMoN GCN (3-layer) Trainium2 kernel over 8 NeuronCores — single SPMD launch.

v3 design: core c owns target nodes [c*12544, (c+1)*12544) (98 blocks of
128).  Per layer: PE transform of own nodes into a narrow bf16 table ->
4-segment 8-way AllGather into per-segment tensors (fired as transform
streams, overlapping aggregation of the previous layer) -> local expand
DMAs place segments into the 256B-strided gather table -> aggregation by
block-group x source-pair: dma_gather of source rows (int16 indices local
to a 25088-row pair slice, 4 SWDGE queues round-robin for parallel Q7
descriptor emission), host-built weighted one-hot masks DMA'd in a
partition-major contiguous layout, PE accumulates exact block segment sums
in PSUM, DVE folds them into an SBUF accumulator.  Per-block combine
(+skip+bias, selu) feeds the next layer's transform directly.  Final
layer: transpose + softmax.  gcn_norm weights (incl. self loops) are baked
into the masks on the host.
"""

import numpy as np
import ml_dtypes

BF16 = ml_dtypes.bfloat16

N = 100000
IN_DIM = 256
HID = 128
HID2 = 64
K = 16

NCORE = 8
NOWN = 12544             # 98*128 padded nodes per core
NPAIR = 2 * NOWN         # 25088 rows per pair slice (< 32768 -> int16)
NALL = 8 * NOWN
NBLK = 98                # target blocks per core
NPH = 4                  # source pair phases
BG = 7                   # blocks per gather group
NGRP = NBLK // BG        # 14
SEG = [0, 25, 50, 74, NBLK]   # AllGather segments (block ranges)

FOUT = [HID, HID2, K]
FIN = [IN_DIM, HID, HID2]

SELU_L = 1.0507009873554805
SELU_A = 1.6732632423543772

_CACHE = {}


def _pad_id(r):
    c = r // 12500
    return c * NOWN + (r - c * 12500)


def _build_plan(edge_index, edge_weight):
    row = np.asarray(edge_index[0], dtype=np.int64)
    col = np.asarray(edge_index[1], dtype=np.int64)
    w = np.asarray(edge_weight, dtype=np.float64)

    deg = np.zeros(N, np.float64)
    np.add.at(deg, col, w)
    deg += 1.0
    dinv = 1.0 / np.sqrt(deg)

    loop = np.arange(N, dtype=np.int64)
    r_all = np.concatenate([row, loop])
    c_all = np.concatenate([col, loop])
    nw = np.concatenate([dinv[row] * w * dinv[col], dinv * dinv])

    rp = _pad_id(r_all)
    cp = _pad_id(c_all)
    tcore = cp // NOWN

    per_core = []
    cnts = np.zeros((NCORE, NBLK, NPH), np.int64)
    for c in range(NCORE):
        sel = tcore == c
        tp = cp[sel] - c * NOWN
        blk = tp // 128
        toff = tp % 128
        pair = rp[sel] // NPAIR
        sloc = rp[sel] - pair * NPAIR
        ew = nw[sel]
        per_core.append((blk, toff, pair, sloc, ew))
        np.add.at(cnts, (c, blk, pair), 1)

    nsub = np.maximum(
        (cnts.max(axis=0) + 127) // 128, 1).astype(np.int64)  # [NBLK, NPH]

    # global sub order: group-major, pair, block, sub-within-run
    sub_blocks, sub_start, sub_stop = [], [], []
    chunk_list = []                      # (pair, sub_off, nsubs) per gather
    run_sub_off = np.zeros((NBLK, NPH), np.int64)
    S = 0
    for g in range(NGRP):
        bs = range(g * BG, min((g + 1) * BG, NBLK))
        for p in range(NPH):
            ch_off = S
            for b in bs:
                run_sub_off[b, p] = S
                k = int(nsub[b, p])
                for i in range(k):
                    sub_blocks.append(b)
                    sub_start.append(i == 0)
                    sub_stop.append(i == k - 1)
                    S += 1
            chunk_list.append((p, ch_off, S - ch_off))
    TOT = S

    NRUN = NBLK * NPH
    run_id = np.arange(NRUN).reshape(NBLK, NPH)
    idx_all, msk_all = [], []
    for c in range(NCORE):
        blk, toff, pair, sloc, ew = per_core[c]
        rid = run_id[blk, pair]
        order = np.argsort(rid, kind="stable")
        rid_s = rid[order]
        rc_cnt = np.bincount(rid_s, minlength=NRUN)
        first = np.concatenate([[0], np.cumsum(rc_cnt)])[:-1]
        within = np.arange(len(rid_s)) - first[rid_s]
        pos = run_sub_off.reshape(-1)[rid_s] * 128 + within
        assert (within < nsub.reshape(-1)[rid_s] * 128).all()

        sidx = np.zeros(TOT * 128, np.int16)
        stof = np.zeros(TOT * 128, np.int64)
        sw = np.zeros(TOT * 128, np.float32)
        sidx[pos] = sloc[order].astype(np.int16)
        stof[pos] = toff[order]
        sw[pos] = ew[order]

        # idx wrapped in 16 partitions, replicated x8 -> [128, TOT*8]
        ii = np.arange(TOT * 128)
        t16 = np.zeros((16, TOT * 8), np.int16)
        t16[ii % 16, ii // 16] = sidx
        idx_all.append(np.tile(t16, (8, 1)))

        # masks, partition-major: msk[p, S, t] = w  iff  toff == t
        msk = np.zeros((128, TOT, 128), np.float32)
        sl = np.arange(TOT * 128)
        msk[sl % 128, sl // 128, stof] = sw
        msk_all.append(msk.astype(BF16))

    return dict(TOT=TOT, chunk_list=chunk_list,
                sub_blocks=np.array(sub_blocks),
                sub_start=np.array(sub_start), sub_stop=np.array(sub_stop),
                nsub=nsub, idx=idx_all, msk=msk_all)


def _build_program(plan):
    import concourse.bacc as bacc
    import concourse.mybir as mybir
    from concourse import tile

    nc = bacc.Bacc("TRN2", target_bir_lowering=False, debug=False,
                   num_devices=NCORE, num_swdge_queues=4)
    f32, bf16, i16 = mybir.dt.float32, mybir.dt.bfloat16, mybir.dt.int16
    AL = mybir.AluOpType
    ACT_EXP = mybir.ActivationFunctionType.Exp
    AXX = mybir.AxisListType.X

    TOT = plan["TOT"]
    chunk_list = plan["chunk_list"]
    sub_blocks = plan["sub_blocks"]
    sub_start = plan["sub_start"]
    sub_stop = plan["sub_stop"]
    NSEG = len(SEG) - 1

    t_x = nc.dram_tensor("x", [NOWN, IN_DIM], f32, kind="ExternalInput")
    t_idx = nc.dram_tensor("idxs", [128, TOT * 8], i16, kind="ExternalInput")
    t_msk = nc.dram_tensor("msk", [128, TOT, 128], bf16,
                           kind="ExternalInput")
    t_W0 = nc.dram_tensor("W0p", [2, 128, HID], f32, kind="ExternalInput")
    t_P0 = nc.dram_tensor("P0p", [2, 128, HID], f32, kind="ExternalInput")
    t_W1 = nc.dram_tensor("W1", [HID, HID2], f32, kind="ExternalInput")
    t_P1 = nc.dram_tensor("P1", [HID, HID2], f32, kind="ExternalInput")
    t_W2 = nc.dram_tensor("W2", [HID2, K], f32, kind="ExternalInput")
    t_bc0 = nc.dram_tensor("bc0", [HID, 1], f32, kind="ExternalInput")
    t_bc1 = nc.dram_tensor("bc1", [HID2, 1], f32, kind="ExternalInput")
    t_bc2 = nc.dram_tensor("bc2", [K, 1], f32, kind="ExternalInput")
    t_id = nc.dram_tensor("ident", [128, 128], f32, kind="ExternalInput")
    t_out = nc.dram_tensor("out", [NOWN, K], f32, kind="ExternalOutput")

    FT = FOUT  # narrow table widths per layer
    d_seg = [[nc.dram_tensor(f"seg{l}_{s}",
                             [NCORE * (SEG[s + 1] - SEG[s]) * 128, FT[l]],
                             bf16, kind="Internal")
              for s in range(NSEG)] for l in range(3)]

    with tile.TileContext(nc) as tc:
        with (
            tc.tile_pool(name="dram", bufs=1, space="DRAM") as dpool,
            tc.tile_pool(name="wts", bufs=1) as wpool,
            tc.tile_pool(name="work", bufs=3) as pool,
            tc.tile_pool(name="ps", bufs=2, space="PSUM") as ppool,
            tc.tile_pool(name="acc", bufs=2, space="PSUM") as apool,
        ):
            d_tab = [dpool.tile([NOWN, FT[l]], bf16, tag=f"tab{l}",
                                name=f"tab{l}") for l in range(3)]
            d_all = [dpool.tile([NALL, 128], bf16, tag=f"all{l}",
                                name=f"all{l}") for l in range(3)]
            d_skT = [dpool.tile([FOUT[l], NOWN], f32, tag=f"skT{l}",
                                name=f"skT{l}") for l in range(2)]

            idt = wpool.tile([128, 128], f32)
            nc.sync.dma_start(idt[:], t_id[:])
            sb_acc = wpool.tile([128, NOWN], f32)

            wW0, wP0 = [], []
            for j in range(2):
                wt = wpool.tile([128, HID], f32, name=f"w0_{j}")
                nc.sync.dma_start(wt[:], t_W0[j])
                wW0.append(wt)
                pt_ = wpool.tile([128, HID], f32, name=f"p0_{j}")
                nc.sync.dma_start(pt_[:], t_P0[j])
                wP0.append(pt_)
            wW1 = wpool.tile([HID, HID2], f32)
            nc.sync.dma_start(wW1[:], t_W1[:])
            wP1 = wpool.tile([HID, HID2], f32)
            nc.sync.dma_start(wP1[:], t_P1[:])
            wW2 = wpool.tile([HID2, K], f32)
            nc.sync.dma_start(wW2[:], t_W2[:])
            wb0 = wpool.tile([HID, 1], f32)
            nc.sync.dma_start(wb0[:], t_bc0[:])
            wb1 = wpool.tile([HID2, 1], f32)
            nc.sync.dma_start(wb1[:], t_bc1[:])
            wb2 = wpool.tile([K, 1], f32)
            nc.sync.dma_start(wb2[:], t_bc2[:])

            Ws = [wW0, [wW1], [wW2]]
            Ps = [wP0, [wP1], None]
            Bs = [wb0, wb1, wb2]

            def fire_ag(l, s):
                rs, re = SEG[s] * 128, SEG[s + 1] * 128
                nr = re - rs
                nc.gpsimd.collective_compute(
                    "AllGather", AL.bypass,
                    replica_groups=[[0, 1, 2, 3, 4, 5, 6, 7]],
                    ins=[d_tab[l][rs:re, :].opt()],
                    outs=[d_seg[l][s][:, :].opt()])
                # expand: place each core's block into the strided table
                for c8 in range(NCORE):
                    nc.sync.dma_start(
                        d_all[l][c8 * NOWN + rs:c8 * NOWN + re, :FT[l]],
                        d_seg[l][s][c8 * nr:(c8 + 1) * nr, :])

            def transform_tile(l, t, xTj):
                fout = FOUT[l]
                pm = ppool.tile([128, fout], f32, tag="pmm")
                nj = len(xTj)
                for j, xj in enumerate(xTj):
                    nc.tensor.matmul(pm[:], xj, Ws[l][j][:],
                                     start=(j == 0), stop=(j == nj - 1))
                tb = pool.tile([128, FT[l]], bf16, tag="tab")
                nc.vector.tensor_copy(tb[:], pm[:])
                nc.sync.dma_start(d_tab[l][t * 128:(t + 1) * 128, :], tb[:])
                if l < 2:
                    ps = ppool.tile([fout, 128], f32, tag="psk")
                    for j, xj in enumerate(xTj):
                        nc.tensor.matmul(ps[:], Ps[l][j][:], xj,
                                         start=(j == 0), stop=(j == nj - 1))
                    sk = pool.tile([fout, 128], f32, tag="skw")
                    nc.vector.tensor_scalar(sk[:], ps[:], Bs[l][:], None,
                                            AL.add)
                    nc.sync.dma_start(d_skT[l][:, t * 128:(t + 1) * 128],
                                      sk[:])

            def combine_block(l, b):
                fout = FOUT[l]
                sl = sb_acc[:fout, b * 128:(b + 1) * 128]
                zz = pool.tile([fout, 128], f32, tag=f"zz{l}")
                if l < 2:
                    skt = pool.tile([fout, 128], f32, tag="skld")
                    nc.sync.dma_start(skt[:],
                                      d_skT[l][:, b * 128:(b + 1) * 128])
                    nc.vector.tensor_tensor(zz[:], sl, skt[:], AL.add)
                else:
                    nc.vector.tensor_scalar(zz[:], sl, Bs[2][:], None,
                                            AL.add)
                ex = pool.tile([fout, 128], f32, tag=f"ex{l}")
                nc.vector.tensor_scalar_min(ex[:], zz[:], 0.0)
                e2 = pool.tile([fout, 128], f32, tag=f"e2{l}")
                nc.scalar.activation(e2[:], ex[:], ACT_EXP)
                nc.vector.tensor_scalar(e2[:], e2[:], SELU_L * SELU_A,
                                        -SELU_L * SELU_A, AL.mult, AL.add)
                nc.vector.tensor_scalar(zz[:], zz[:], 0.0, SELU_L,
                                        AL.max, AL.mult)
                nc.vector.tensor_tensor(zz[:], zz[:], e2[:], AL.add)
                if l < 2:
                    transform_tile(l + 1, b, [zz[:]])
                else:
                    pt2 = ppool.tile([128, K], f32, tag="psk")
                    nc.tensor.matmul(pt2[:], zz[:], idt[:K, :K],
                                     start=True, stop=True)
                    sm = pool.tile([128, K], f32, tag="sms")
                    mx = pool.tile([128, 1], f32, tag="mx")
                    nc.vector.tensor_reduce(mx[:], pt2[:], AXX, AL.max)
                    nc.vector.tensor_scalar(sm[:], pt2[:], mx[:], None,
                                            AL.subtract)
                    nc.scalar.activation(sm[:], sm[:], ACT_EXP)
                    s2 = pool.tile([128, 1], f32, tag="s2")
                    nc.vector.tensor_reduce(s2[:], sm[:], AXX, AL.add)
                    rc = pool.tile([128, 1], f32, tag="rc")
                    nc.vector.reciprocal(rc[:], s2[:])
                    nc.vector.tensor_scalar(sm[:], sm[:], rc[:], None,
                                            AL.mult)
                    nc.sync.dma_start(t_out[b * 128:(b + 1) * 128, :], sm[:])

            # ---- layer 0 transform (standalone) + segmented AG(0) ----
            seg_after_tile = {SEG[s + 1] - 1: s for s in range(NSEG)}
            for t in range(NBLK):
                xin = pool.tile([128, IN_DIM], f32, tag="xin")
                nc.sync.dma_start(xin[:], t_x[t * 128:(t + 1) * 128, :])
                xT = pool.tile([128, IN_DIM], f32, tag="xT")
                for j in range(2):
                    ptr = ppool.tile([128, 128], f32, tag="ptr")
                    nc.tensor.transpose(ptr[:],
                                        xin[:, j * 128:(j + 1) * 128],
                                        idt[:])
                    nc.vector.tensor_copy(xT[:, j * 128:(j + 1) * 128],
                                          ptr[:])
                transform_tile(0, t, [xT[:, 0:128], xT[:, 128:256]])
                if t in seg_after_tile:
                    fire_ag(0, seg_after_tile[t])

            seg_after_group = {}
            for s in range(NSEG):
                g = (SEG[s + 1] + BG - 1) // BG - 1
                seg_after_group.setdefault(g, []).append(s)

            # ---- layers ----
            for l in range(3):
                fout = FOUT[l]
                acc = None
                for g in range(NGRP):
                    # batched idx load for the group's 4 chunks
                    g_off = chunk_list[g * NPH][1]
                    g_end = chunk_list[g * NPH + NPH - 1][1] + \
                        chunk_list[g * NPH + NPH - 1][2]
                    git = pool.tile([128, (g_end - g_off) * 8], i16,
                                    tag="idxt")
                    nc.sync.dma_start(
                        git[:], t_idx[:, g_off * 8:g_end * 8])
                    for p in range(NPH):
                        pair, ch_off, kk = chunk_list[g * NPH + p]
                        gth = pool.tile([128, kk, 128], bf16, tag="gath")
                        qn = (g * NPH + p) % 4
                        nc.gpsimd.dma_gather(
                            gth[:], d_all[l][pair * NPAIR:(pair + 1) * NPAIR,
                                             :],
                            git[:, (ch_off - g_off) * 8:
                                (ch_off - g_off + kk) * 8],
                            kk * 128, kk * 128, 128,
                            single_packet=False, queue_num=qn)
                        mkc = pool.tile([128, kk, 128], bf16, tag="mskc")
                        nc.sync.dma_start(
                            mkc[:], t_msk[:, ch_off:ch_off + kk, :])
                        for j in range(kk):
                            sub = ch_off + j
                            if sub_start[sub]:
                                acc = apool.tile([fout, 128], f32,
                                                 tag="accp",
                                                 name=f"acc{l}_{sub}")
                            nc.tensor.matmul(acc[:], gth[:, j, :fout],
                                             mkc[:, j, :],
                                             start=bool(sub_start[sub]),
                                             stop=bool(sub_stop[sub]))
                            if sub_stop[sub]:
                                b = int(sub_blocks[sub])
                                dst = sb_acc[:fout,
                                             b * 128:(b + 1) * 128]
                                if p == 0:
                                    nc.vector.tensor_copy(dst, acc[:])
                                else:
                                    nc.vector.tensor_tensor(dst, dst,
                                                            acc[:], AL.add)
                    for b in range(g * BG, min((g + 1) * BG, NBLK)):
                        combine_block(l, b)
                    if l < 2 and g in seg_after_group:
                        for s in seg_after_group[g]:
                            fire_ag(l + 1, s)
    nc.compile()
    return nc


def _get_compiled(inputs):
    k = "prog"
    if k not in _CACHE:
        plan = _build_plan(inputs["edge_index"], inputs["edge_weight"])
        nc = _build_program(plan)
        _CACHE[k] = (plan, nc)
    return _CACHE[k]


def kernel(_trace=False, **inputs):
    from concourse.bass_utils import run_bass_kernel_spmd

    plan, nc = _get_compiled(inputs)

    x = np.asarray(inputs["x"], np.float32)
    xpad = np.zeros((NCORE, NOWN, IN_DIM), np.float32)
    for c in range(NCORE):
        xpad[c, :12500] = x[c * 12500:(c + 1) * 12500]

    def wchunks(W, n):
        out = np.zeros((n, 128, W.shape[1]), np.float32)
        for j in range(n):
            out[j, :min(128, W.shape[0] - j * 128)] = \
                W[j * 128:(j + 1) * 128]
        return out

    W0p = wchunks(np.asarray(inputs["W0"], np.float32), 2)
    P0p = wchunks(np.asarray(inputs["P0w"], np.float32), 2)
    W1 = np.asarray(inputs["W1"], np.float32)
    P1 = np.asarray(inputs["P1w"], np.float32)
    W2 = np.asarray(inputs["W2"], np.float32)
    bc0 = (np.asarray(inputs["b0"]) + np.asarray(inputs["P0b"])) \
        .astype(np.float32).reshape(HID, 1)
    bc1 = (np.asarray(inputs["b1"]) + np.asarray(inputs["P1b"])) \
        .astype(np.float32).reshape(HID2, 1)
    bc2 = np.asarray(inputs["b2"]).astype(np.float32).reshape(K, 1)
    ident = np.eye(128, dtype=np.float32)

    in_maps = []
    for c in range(NCORE):
        in_maps.append({
            "x": xpad[c],
            "idxs": plan["idx"][c], "msk": plan["msk"][c],
            "W0p": W0p, "P0p": P0p, "W1": W1, "P1": P1, "W2": W2,
            "bc0": bc0, "bc1": bc1, "bc2": bc2,
            "ident": ident,
        })
    res = run_bass_kernel_spmd(nc, in_maps, core_ids=list(range(NCORE)),
                               trace=_trace)
    if _trace:
        kernel.last_exec_ns = res.exec_time_ns
        kernel.last_profile_json = res.profile_json
    out = np.zeros((N, K), np.float32)
    for c in range(NCORE):
        out[c * 12500:(c + 1) * 12500] = res.results[c]["out"][:12500]
    return out
